# revision 5
# baseline (speedup 1.0000x reference)
"""Trainium2 Bass kernel for nn_CWTLayer: CWT (mexh, 128 scales) + bilinear
resize (B,128,4096,C) -> (B,224,224,C).

v3 = v2 (decimated folded-matmul, bf16) + per-batch pipeline.

Conv free dim is quartered by batch (rows r = b*C + c, 3 rows per batch), and
the scale-resize + output DMA for batch k overlap the conv of batch k+1, so
the 2.4MB/core output stream hides under PE work instead of being a serial
tail.  ot uses a phase-major column order (widx = p*32 + q, w = 7q + p) so
PSUM->SBUF conv copies write contiguous runs; the resize copies undo the
permutation with a 2-D strided AP at no extra cost (they were strided
already).  Edge-column conv groups run right after batch 0's quarter.

See kernel_v2.py docstring for the decimation scheme itself.
"""

import numpy as np
import ml_dtypes

# ---------------- static configuration ----------------
B, T, C = 32, 4096, 3
SCALES = 128
KLF = 10
OUT_H, OUT_W = 224, 224
N_PSI = 4096
SPAN = 16.0
STEP = SPAN / (N_PSI - 1)
MAX_SCALE = T // (2 * KLF)
SCALES_VEC = np.logspace(np.log10(2.0), np.log10(MAX_SCALE), SCALES).astype(np.float32)
PAD = min((N_PSI - 1) // 2, T - 1)          # 2047
TP = T + 2 * PAD                            # 8190
TP2 = -(-TP // 128) * 128                   # 8192
NPHASE = 7
QN = OUT_W // NPHASE                        # 32
N_CORES = 8
BPC = B // N_CORES                          # 4 batches per core
RPC = C * BPC                               # 12 rows per core

PHASE_D = (2, 4, 8, 32)                     # per 32-scale block, interior rows
EDGE_D = (1, 1, 2, 2)                       # per block, w=0 / w=223 rows
NBLK = 4
BW = 32                                     # scales per block

J_LIST = []
for _s in SCALES_VEC:
    _sf = float(_s)
    _n = int(np.ceil(_sf * SPAN + 1.0))
    _j = np.floor(np.arange(_n, dtype=np.float64) / (_sf * STEP)).astype(np.int32)
    J_LIST.append(np.clip(_j, 0, N_PSI - 1))
KS = [len(j) for j in J_LIST]
DS = [max((TP - k - T) // 2, 0) for k in KS]


def _resize_weight_mat(in_size, out_size):
    """jax.image.resize bilinear (antialias=True) weight matrix, float64."""
    scale = out_size / in_size
    inv_scale = 1.0 / scale
    kernel_scale = max(inv_scale, 1.0)
    sample_f = (np.arange(out_size, dtype=np.float64) + 0.5) * inv_scale - 0.5
    x = np.abs(sample_f[:, None] - np.arange(in_size, dtype=np.float64)[None, :]) / kernel_scale
    w = np.maximum(0.0, 1.0 - x)
    total = w.sum(axis=1, keepdims=True)
    w = np.where(np.abs(total) > 1000.0 * np.finfo(np.float32).eps, w / total, 0.0)
    ok = (sample_f >= -0.5) & (sample_f <= in_size - 0.5)
    return np.where(ok[:, None], w, 0.0)


_WT = _resize_weight_mat(T, OUT_W)          # (224, 4096)
_A = _resize_weight_mat(SCALES, OUT_H)      # (224, 128)


def _kernel_rows():
    rows = []
    for p in range(NPHASE):
        rows.append(dict(kind="phase", p=p, w_row=(p if p != 0 else 7),
                         shift=(128 if p == 0 else 0)))
    for wrow in (0, OUT_W - 1):
        rows.append(dict(kind="edge", p=None, w_row=wrow, shift=0))
    return rows


_KROWS = _kernel_rows()

# ---- group geometry (shape-only; no psi values) ----
_GROUPS = []
_acc = 0
for _ki, _kr in enumerate(_KROWS):
    _t_nz = np.nonzero(_WT[_kr["w_row"]])[0]
    _t0, _t1 = int(_t_nz[0]), int(_t_nz[-1])
    for _blk in range(NBLK):
        _D = (PHASE_D if _kr["kind"] == "phase" else EDGE_D)[_blk]
        _mlo, _mhi = 10**9, -10**9
        for _s in range(_blk * BW, _blk * BW + BW):
            _A_s = _t0 + DS[_s] - _kr["shift"]
            _len = (_t1 - _t0 + 1) + KS[_s] + 1
            _mlo = min(_mlo, -(-_A_s // _D))
            _mhi = max(_mhi, (_A_s + _len - 1) // _D)
        _align = 128 // _D
        _mbase = (_mlo // _align) * _align
        _nck = -(-(_mhi + 1 - _mbase) // 128)
        _GROUPS.append(dict(ki=_ki, blk=_blk, D=_D, nck=_nck, mbase=_mbase,
                            j0=_mbase * _D // 128, kind=_kr["kind"],
                            p=_kr.get("p"), w_row=_kr["w_row"],
                            shift=_kr["shift"], t0=_t0, t1=_t1, goff=_acc))
        _acc += _nck * BW
_FTOT = _acc

LEVELS = sorted({g["D"] for g in _GROUPS})
_LJ = {}
for _D in LEVELS:
    _jlo, _jhi = 10**9, -10**9
    for g in _GROUPS:
        if g["D"] != _D:
            continue
        _nf = QN if g["kind"] == "phase" else 1
        _jlo = min(_jlo, g["j0"])
        _jhi = max(_jhi, g["j0"] + (g["nck"] - 1) * _D + _nf)
    _jw = _jhi - _jlo
    _jw += _jw % 2                           # even word count (4B-mult row stride)
    _LJ[_D] = (_jlo, _jw)

_BF16 = ml_dtypes.bfloat16


def _build_wall(int_psi_base):
    psi = np.asarray(int_psi_base, dtype=np.float64)
    wall = np.zeros((128, _FTOT), dtype=np.float64)
    for g in _GROUPS:
        D = g["D"]
        wt_row = _WT[g["w_row"], g["t0"]: g["t1"] + 1]
        n = g["nck"] * 128
        m_abs = g["mbase"] + np.arange(n)
        for si in range(BW):
            sidx = g["blk"] * BW + si
            kern = psi[J_LIST[sidx]][::-1]
            kpad = np.concatenate([[0.0], kern, [0.0]])
            dk = -np.sqrt(np.float64(SCALES_VEC[sidx])) * (kpad[:-1] - kpad[1:])
            h = np.convolve(wt_row, dk)
            A_s = g["t0"] + DS[sidx] - g["shift"]
            idx = m_abs * D - A_s
            ok = (idx >= 0) & (idx < len(h))
            col = np.zeros(n)
            col[ok] = h[idx[ok]]
            cv = col.reshape(g["nck"], 128)            # [c, kk]
            wall[:, g["goff"] + si: g["goff"] + g["nck"] * BW: BW] = cv.T
    return np.ascontiguousarray(wall.astype(_BF16))


def _keys_taps(D, a=-0.5):
    i = np.arange(-2 * D + 1, 2 * D)
    x = np.abs(i) / D
    w = np.where(x < 1, (a + 2) * x**3 - (a + 3) * x**2 + 1,
                 np.where(x < 2, a * x**3 - 5 * a * x**2 + 8 * a * x - 4 * a, 0.0))
    return w


def _build_levels(xp):
    """xp (RPC, TP) f32 -> {D: X_D (128, RPC*JW_D) bf16}."""
    out = {}
    for D in LEVELS:
        jlo, JW = _LJ[D]
        st = 128 // D
        md_need = max((jlo + JW) * st + 128, TP2 // D + 8)
        if D == 1:
            xD = np.zeros((RPC, md_need), np.float64)
            xD[:, :TP] = xp
        else:
            taps = _keys_taps(D)
            lpad = 2 * D - 1
            xpad = np.pad(xp.astype(np.float64), ((0, 0), (lpad, len(taps) + D)))
            from numpy.lib.stride_tricks import sliding_window_view
            sw = sliding_window_view(xpad, len(taps), axis=1)   # (r, L, taps)
            mmax = min(md_need, (sw.shape[1] - 1) // D + 1)
            xD = np.zeros((RPC, md_need), np.float64)
            xD[:, :mmax] = sw[:, : mmax * D: D, :] @ taps
        pos = (jlo + np.arange(JW))[None, :] * st + np.arange(128)[:, None]
        X = xD[:, pos]                               # (r, 128, JW)
        X = np.ascontiguousarray(X.transpose(1, 0, 2).reshape(128, RPC * JW))
        out[D] = X.astype(_BF16)
    return out


def _pad_rows(x):
    """(B,T,C) -> (CB, TP) reflect-padded, b-major rows r = b*C+c."""
    xp = np.pad(x, ((0, 0), (PAD, PAD), (0, 0)), mode="reflect")
    return np.transpose(xp, (0, 2, 1)).reshape(B * C, TP)


# ---------------- bass program ----------------
_NC_CACHE = {}

_DMA_ORDER_LV = {2: 0, 4: 2, 8: 3, 16: 4, 1: 9}   # interleave with weights


def _get_nc():
    if "nc" in _NC_CACHE:
        return _NC_CACHE["nc"]
    import concourse.bacc as bacc
    import concourse.mybir as mybir
    from concourse import tile

    f32 = mybir.dt.float32
    bf16 = mybir.dt.bfloat16
    nc = bacc.Bacc(None)
    xl_d = {D: nc.dram_tensor(f"xl{D}", [128, RPC * _LJ[D][1]], bf16,
                              kind="ExternalInput") for D in LEVELS}
    wall_d = nc.dram_tensor("wall", [128, _FTOT], bf16, kind="ExternalInput")
    at_d = nc.dram_tensor("at", [64, 2 * OUT_H], bf16, kind="ExternalInput")
    out_d = nc.dram_tensor("out", [BPC, OUT_H, OUT_W, C], f32, kind="ExternalOutput")

    NKI = len(_KROWS)

    with tile.TileContext(nc) as tc:
        with (
            tc.tile_pool(name="const", bufs=1) as cpool,
            tc.tile_pool(name="w", bufs=1) as wpool,
            tc.tile_pool(name="ot", bufs=1) as otpool,
            tc.tile_pool(name="psph", bufs=4, space="PSUM") as psph,
            tc.tile_pool(name="psed", bufs=2, space="PSUM") as psed,
            tc.tile_pool(name="ps2", bufs=2, space="PSUM") as ps2pool,
            tc.tile_pool(name="res", bufs=1) as respool,
        ):
            # ---- input DMAs, ordered so batch-0 conv can start asap:
            # xl2, w0, xl4, xl8, xl16, w1..w6, xl1, w7, w8, at
            xl_tiles = {}
            for D in LEVELS:
                xl_tiles[D] = cpool.tile([128, RPC * _LJ[D][1]], bf16, tag=f"xl{D}", name=f"xl{D}")
            wt_tiles = {}
            row_off = {}
            for ki in range(NKI):
                gs = [g for g in _GROUPS if g["ki"] == ki]
                c0 = gs[0]["goff"]
                c1 = gs[-1]["goff"] + gs[-1]["nck"] * BW
                wt_tiles[ki] = wpool.tile([128, c1 - c0], bf16, tag=f"w{ki}", name=f"w{ki}")
                row_off[ki] = c0
            at = cpool.tile([64, 2 * OUT_H], bf16)

            def dma_w(ki):
                gs = [g for g in _GROUPS if g["ki"] == ki]
                c0 = gs[0]["goff"]
                c1 = gs[-1]["goff"] + gs[-1]["nck"] * BW
                nc.sync.dma_start(out=wt_tiles[ki][:], in_=wall_d[:, c0:c1])

            def dma_xl(D):
                nc.sync.dma_start(out=xl_tiles[D][:], in_=xl_d[D][:])

            lv_sorted = sorted(LEVELS, key=lambda D: _DMA_ORDER_LV.get(D, 5))
            dma_xl(lv_sorted[0])
            dma_w(0)
            for D in lv_sorted[1:]:
                dma_xl(D)
            for ki in range(1, NPHASE):
                dma_w(ki)
            dma_w(7)
            dma_w(8)
            nc.sync.dma_start(out=at[:], in_=at_d[:])

            xl = {D: xl_tiles[D][:].rearrange("p (r j) -> p r j", r=RPC)
                  for D in LEVELS}

            # ot: phase-major column order widx = p*QN + q  (w = 7q + p).
            # edge cols w=0 -> widx 0, w=223 -> widx 223 (p=6,q=31).
            otA = otpool.tile([64, RPC * OUT_W], bf16)   # scales 0..63
            otB = otpool.tile([64, RPC * OUT_W], bf16)   # scales 64..127
            oeA = otpool.tile([64, 2 * RPC], bf16)       # edge cols, low scales
            oeB = otpool.tile([64, 2 * RPC], bf16)
            ot_pm = [t[:].rearrange("p (r s q) -> p r s q", s=NPHASE, q=QN)
                     for t in (otA, otB)]
            ot_rw = [t[:].rearrange("p (r w) -> p r w", r=RPC)
                     for t in (otA, otB)]
            oes = (oeA, oeB)

            ncopy = [0]

            def copy_eng(dst, src):
                if ncopy[0] % 2 == 0:
                    nc.vector.tensor_copy(dst, src)
                else:
                    nc.scalar.copy(dst, src)
                ncopy[0] += 1

            def conv_quarter(bq):
                """phase-row conv for a half (rows 6h..6h+5)."""
                r0 = bq * 6
                for ki in range(NPHASE):
                    gs = [g for g in _GROUPS if g["ki"] == ki]
                    wt = wt_tiles[ki]
                    for g in gs:
                        ps = psph.tile([BW, 6 * QN], f32, tag="ph", name="ph")
                        D, nck = g["D"], g["nck"]
                        jj0 = g["j0"] - _LJ[D][0]
                        for c in range(nck):
                            jj = jj0 + c * D
                            rhs = xl[D][:, r0:r0 + 6, jj: jj + QN]
                            nc.tensor.matmul(
                                ps[:],
                                wt[:, g["goff"] - row_off[ki] + c * BW:
                                   g["goff"] - row_off[ki] + (c + 1) * BW],
                                rhs, start=(c == 0), stop=(c == nck - 1))
                        pb = (g["blk"] % 2) * BW
                        dst = ot_pm[g["blk"] // 2][pb:pb + BW, r0:r0 + 6,
                                                   gs[0]["p"], :]
                        psv = ps[:].rearrange("p (r q) -> p r q", r=6)
                        copy_eng(dst, psv)

            def conv_edges():
                """edge-column conv, all rows (N=12); results stay in PSUM
                (edge_ps) and are patched into ot per quarter."""
                for ki in (NPHASE, NPHASE + 1):
                    gs = [g for g in _GROUPS if g["ki"] == ki]
                    wt = wt_tiles[ki]
                    wi_idx = ki - NPHASE
                    for g in gs:
                        ps = psed.tile([BW, RPC], f32, tag="ed", name="ed")
                        D, nck = g["D"], g["nck"]
                        jj0 = g["j0"] - _LJ[D][0]
                        for c in range(nck):
                            rhs = xl[D][:, :, jj0 + c * D]
                            nc.tensor.matmul(
                                ps[:],
                                wt[:, g["goff"] - row_off[ki] + c * BW:
                                   g["goff"] - row_off[ki] + (c + 1) * BW],
                                rhs, start=(c == 0), stop=(c == nck - 1))
                        pb = (g["blk"] % 2) * BW
                        copy_eng(oes[g["blk"] // 2][pb:pb + BW,
                                 wi_idx * RPC:(wi_idx + 1) * RPC], ps[:])

            def patch_edges(bq):
                r0 = bq * 6
                for wi_idx, wi in ((0, 0), (1, OUT_W - 1)):
                    for half in range(2):
                        copy_eng(ot_rw[half][:, r0:r0 + 6, wi],
                                 oes[half][:, wi_idx * RPC + r0:
                                           wi_idx * RPC + r0 + 6])

            # resize weights: out h partitions; ps2 448-col for rows (r0,r0+1),
            # ps3 224-col for row r0+2; res copies un-permute widx -> w.
            out_hb = out_d[:].rearrange("b h w c -> h b (w c)")
            res = {}
            for hb, hm in ((0, 128), (1, OUT_H - 128)):
                res[hb] = respool.tile([128, BPC * OUT_W * C], f32, tag=f"res{hb}", name=f"res{hb}")

            def resize_quarter(bq):
                r0 = bq * C
                for hb, hm in ((0, 128), (1, OUT_H - 128)):
                    res_q = res[hb][:].rearrange("p (b w c) -> p b w c",
                                                 b=BPC, w=OUT_W)
                    for mi, (rr0, nrow) in enumerate(((0, 2), (2, 1))):
                        ncols = nrow * OUT_W
                        ps2 = ps2pool.tile([128, 2 * OUT_W], f32, tag="ps2")
                        c0_, c1_ = (r0 + rr0) * OUT_W, (r0 + rr0 + nrow) * OUT_W
                        nc.tensor.matmul(
                            ps2[:hm, :ncols], at[:, hb * 128: hb * 128 + hm],
                            otA[:, c0_:c1_], start=True, stop=False)
                        nc.tensor.matmul(
                            ps2[:hm, :ncols],
                            at[:, OUT_H + hb * 128: OUT_H + hb * 128 + hm],
                            otB[:, c0_:c1_], start=False, stop=True)
                        for rr in range(nrow):
                            r = r0 + rr0 + rr
                            cc = r - r0
                            # src widx = p*32+q, dst w = 7q+p
                            src = ps2[:hm, rr * OUT_W:(rr + 1) * OUT_W]
                            src2 = src.rearrange("h (s q) -> h s q", s=NPHASE)
                            dst = res_q[:hm, bq, :, cc]
                            dst2 = dst.rearrange("h (q s) -> h s q", s=NPHASE)
                            copy_eng(dst2, src2)
                    nc.sync.dma_start(
                        out=out_hb[hb * 128: hb * 128 + hm, bq:bq + 1, :],
                        in_=res[hb][:hm, bq * OUT_W * C:(bq + 1) * OUT_W * C]
                        .rearrange("p (b f) -> p b f", b=1))

            conv_quarter(0)
            conv_edges()
            patch_edges(0)
            resize_quarter(0)
            resize_quarter(1)
            conv_quarter(1)
            patch_edges(1)
            resize_quarter(2)
            resize_quarter(3)

    nc.finalize()
    _NC_CACHE["nc"] = nc
    return nc


def _prepare_in_maps(x, int_psi_base):
    x = np.asarray(x, dtype=np.float32)
    wall = _build_wall(int_psi_base)
    atm = np.ascontiguousarray(np.concatenate(
        [_A.T[:64], _A.T[64:]], axis=1).astype(np.float32)).astype(_BF16)
    xp_all = _pad_rows(x)                                    # (B*C, TP)

    in_maps = []
    for core in range(N_CORES):
        rows = slice((core * BPC) * C, (core * BPC + BPC) * C)
        xpc = np.ascontiguousarray(xp_all[rows]).astype(np.float32)
        lv = _build_levels(xpc)
        m = {f"xl{D}": lv[D] for D in LEVELS}
        m["wall"] = wall
        m["at"] = atm
        in_maps.append(m)
    return in_maps


def _run(x, int_psi_base, **spmd_kwargs):
    from concourse.bass_utils import run_bass_kernel_spmd

    in_maps = _prepare_in_maps(x, int_psi_base)
    nc = _get_nc()
    res = run_bass_kernel_spmd(nc, in_maps, list(range(N_CORES)), **spmd_kwargs)
    out = np.concatenate([res.results[i]["out"] for i in range(N_CORES)], axis=0)
    return out, res


def kernel(x, int_psi_base):
    return _run(x, int_psi_base)[0]

